# revision 18
# baseline (speedup 1.0000x reference)
"""Causal no-head self-attention with RoPE on 8 Trainium2 NeuronCores.

Sharding: 8 cores = 4 batches x 2 query-sets of four 256-query blocks.
Per-core slots s=0..3 run 4(s+1) key-tile visits (128 keys each); block
assignment (h=0: blocks {1,3,4,7}, h=1: {0,2,5,6}) makes the same
(4,8,12,16) visit structure causally sufficient on every core, so all
cores run ONE identical Bass program; per-core differences (which
queries, which keys, causal masks, RoPE angles) are carried in the
input data.

K/V projection is deduplicated across the two cores sharing a batch:
core h projects (and RoPEs) keys [512h,512h+512) u [1024+512h,...+512)
only, and the pair exchanges 512-key blocks with two in-pair
AllGathers (AG#0 -> seq blocks 0,1; AG#1 -> blocks 2,3) that overlap
the Q projection. Everything is bf16 (PE full rate, half DMA/SBUF of
fp32) with fp32 PSUM accumulation; K^T/V/Q^T live entirely in SBUF.
QT/KT are [d_k, cols] with d_k de-interleaved (even feats rows 0:512,
odd 512:1024) so RoPE is a contiguous-partition-block rotation; the
same permutation is applied to Wq/Wk output columns on host (scores
are invariant). QT doubles as attnT storage.
"""

import numpy as np
import sys

for _p in ("/opt/trn_rl_repo",):
    if _p not in sys.path:
        sys.path.insert(0, _p)

import concourse.bass as bass
import concourse.bacc as bacc
import concourse.mybir as mybir
from concourse.tile import TileContext
from concourse.bass_utils import run_bass_kernel_spmd

B, S, D = 4, 2048, 1024
THETA = 10000.0
P = 128
NT = D // P          # 8 partition-tiles over the 1024 dim
SB = 512             # seq block width for K/V projection
QB = 256             # query slot width (4 slots per core)
NQ = 1024            # queries (and local keys) per core
F32 = mybir.dt.float32
F32R = mybir.dt.float32r
BF16 = mybir.dt.bfloat16
SCALE = 1.0 / 32.0   # 1/sqrt(D)
BLOCKS = [[1, 3, 4, 7], [0, 2, 5, 6]]   # 256-query blocks per core half
NVIS = [4, 8, 12, 16]                    # kt visits per slot (same all cores)


def _build_program():
    nc = bacc.Bacc("TRN2", num_swdge_queues=4)
    inp = {}
    def din(name, shape, dt):
        inp[name] = nc.dram_tensor(name, shape, dt, kind="ExternalInput")
    din("xTm", [D, NQ], BF16)
    din("xTq", [D, NQ], BF16)
    din("WqT", [D, D], BF16)
    din("WkT", [D, D], BF16)
    din("WvT", [D, D], BF16)
    din("WoT", [D, D], BF16)
    din("cosM", [D // 2, NQ], BF16)
    din("sinM", [D // 2, NQ], BF16)
    din("cosQ", [D // 2, NQ], BF16)
    din("sinQ", [D // 2, NQ], BF16)
    din("masks", [P, 16, QB], BF16)
    din("ones_col", [P, 1], F32R)
    din("ones_row", [1, P], F32R)
    outT = nc.dram_tensor("outT", [P, NT, NQ], BF16, kind="ExternalOutput")

    xTm_r = inp["xTm"].rearrange("(t p) s -> p t s", p=P)
    xTq_r = inp["xTq"].rearrange("(t p) s -> p t s", p=P)
    WqT_r = inp["WqT"].rearrange("(t p) o -> p t o", p=P)
    WkT_r = inp["WkT"].rearrange("(t p) o -> p t o", p=P)
    WvT_r = inp["WvT"].rearrange("(t p) o -> p t o", p=P)
    WoT_r = inp["WoT"].rearrange("(t p) o -> p t o", p=P)
    cosM_r = inp["cosM"].rearrange("(t p) s -> p t s", p=P)
    sinM_r = inp["sinM"].rearrange("(t p) s -> p t s", p=P)
    cosQ_r = inp["cosQ"].rearrange("(t p) s -> p t s", p=P)
    sinQ_r = inp["sinQ"].rearrange("(t p) s -> p t s", p=P)

    from contextlib import ExitStack
    with TileContext(nc) as tc:
        with ExitStack() as ctx:
            pool = lambda *a, **kw: ctx.enter_context(tc.tile_pool(*a, **kw))
            res = pool(name="res", bufs=1)          # big residents
            dpool = pool(name="dram", bufs=1, space="DRAM")
            wres = pool(name="wres", bufs=1)        # weights
            smp = pool(name="small", bufs=1)
            xbp = pool(name="xb", bufs=2)
            csp = pool(name="cs", bufs=2)
            rawp = pool(name="raw", bufs=2)
            tmpp = pool(name="tmp", bufs=1)
            ptp = pool(name="pt", bufs=1)
            sap = pool(name="sa", bufs=1)
            bcp = pool(name="bc", bufs=2)
            obp = pool(name="ob", bufs=1)
            psB = pool(name="psB", bufs=4, space="PSUM")
            psPV = pool(name="psPV", bufs=4, space="PSUM")

            KT = res.tile([P, NT, S], BF16, tag="kt")
            V = res.tile([P, S // P, D], BF16, tag="v")
            # QT doubles as attnT: slot s's normalized PV overwrites QT's
            # columns after the slot's scores are done reading them.
            QT = res.tile([P, NT, NQ], BF16, tag="qt")
            attnT = QT
            maskst = res.tile([P, 16, QB], BF16, tag="msk")
            ones_col = smp.tile([P, 1], F32R, tag="onescol")
            nc.sync.dma_start(ones_col[:], inp["ones_col"][:])
            ones_row = smp.tile([1, P], F32R, tag="onesrow")
            nc.sync.dma_start(ones_row[:], inp["ones_row"][:])

            def rope_block(dst, src, cos_t, sin_t):
                # dst/src: [P, NT, w]; rows 0:NT/2 = even feats, NT/2: = odd
                h = NT // 2
                w = src.shape[-1]
                e, o = src[:, 0:h, :], src[:, h:NT, :]
                c, s = cos_t[:, :, :], sin_t[:, :, :]
                t1 = tmpp.tile([P, h, SB], BF16, tag="t1")
                nc.vector.tensor_mul(out=dst[:, 0:h, :], in0=e, in1=c)
                nc.vector.tensor_mul(out=t1[:, :, 0:w], in0=o, in1=s)
                nc.vector.tensor_tensor(dst[:, 0:h, :], dst[:, 0:h, :],
                                        t1[:, :, 0:w], mybir.AluOpType.subtract)
                t2 = tmpp.tile([P, h, SB], BF16, tag="t1")
                nc.vector.tensor_mul(out=dst[:, h:NT, :], in0=o, in1=c)
                nc.vector.tensor_mul(out=t2[:, :, 0:w], in0=e, in1=s)
                nc.vector.tensor_tensor(dst[:, h:NT, :], dst[:, h:NT, :],
                                        t2[:, :, 0:w], mybir.AluOpType.add)

            # ---------- Phase 0: K/V projection of MY half of the keys ------
            # jb-th local block = global seq block 2*jb + rank; the in-pair
            # AllGather over [rank0 chunk | rank1 chunk] then yields seq
            # blocks {0,1} (AG#0) and {2,3} (AG#1) in natural order.
            RG = [[0, 1], [2, 3], [4, 5], [6, 7]]
            kvin = [dpool.tile([P, 16, SB], BF16, name=f"kvin{j}") for j in range(2)]
            kvout = [dpool.tile([2, P, 16, SB], BF16, name=f"kvout{j}") for j in range(2)]

            xms, cms, sms = [], [], []
            for jb in range(2):
                sl = slice(jb * SB, (jb + 1) * SB)
                xm = xbp.tile([P, NT, SB], BF16, tag="xb")
                for t in range(NT):
                    nc.sync.dma_start(xm[:, t, :], xTm_r[:, t, sl])
                cm = csp.tile([P, NT // 2, SB], BF16, tag="cs_c")
                nc.gpsimd.dma_start(cm[:], cosM_r[:, :, sl])
                sm = csp.tile([P, NT // 2, SB], BF16, tag="cs_s")
                nc.gpsimd.dma_start(sm[:], sinM_r[:, :, sl])
                xms.append(xm); cms.append(cm); sms.append(sm)
                if jb == 0:
                    WkRes = wres.tile([P, NT, D], BF16, tag="wk")
                    for t in range(NT):
                        for hh in range(2):
                            w_sl = slice(hh * SB, (hh + 1) * SB)
                            nc.sync.dma_start(WkRes[:, t, w_sl], WkT_r[:, t, w_sl])
                else:
                    WvRes = wres.tile([P, NT, D], BF16, tag="wv")
                    for t in range(NT):
                        for hh in range(2):
                            w_sl = slice(hh * SB, (hh + 1) * SB)
                            nc.sync.dma_start(WvRes[:, t, w_sl], WvT_r[:, t, w_sl])
            nc.gpsimd.dma_start(maskst[:], inp["masks"][:])

            for jb in range(2):
                sl = slice(jb * SB, (jb + 1) * SB)
                xb = xms[jb]
                kraw = rawp.tile([P, NT, SB], BF16, tag="raw")
                for t_out in range(NT):
                    ps = psB.tile([P, SB], F32, tag="psB")
                    for dt_ in range(NT):
                        nc.tensor.matmul(ps[:], WkRes[:, dt_, t_out * P:(t_out + 1) * P],
                                         xb[:, dt_, :], start=(dt_ == 0), stop=(dt_ == NT - 1))
                    nc.vector.tensor_copy(kraw[:, t_out, :], ps[:])
                rope_block(KT[:, :, sl], kraw, cms[jb], sms[jb])
                for sk_ in range(SB // P):
                    for dh in range(2):
                        ps = psB.tile([P, SB], F32, tag="psB")
                        for dt_ in range(NT):
                            nc.tensor.matmul(ps[:], xb[:, dt_, sk_ * P:(sk_ + 1) * P],
                                             WvRes[:, dt_, dh * SB:(dh + 1) * SB],
                                             start=(dt_ == 0), stop=(dt_ == NT - 1))
                        nc.scalar.copy(V[:, jb * (SB // P) + sk_, dh * SB:(dh + 1) * SB], ps[:])
                # spills issued from scalar right after the V copies, when
                # all the data is already resident -- the descriptors reach
                # the DMA queues without blocking them on waits
                for t in range(NT):
                    nc.scalar.dma_start(kvin[jb][:, t, :], KT[:, t, sl])
                for sk_ in range(SB // P):
                    for dh in range(2):
                        nc.scalar.dma_start(kvin[jb][:, 8 + 2 * sk_ + dh, :],
                                            V[:, jb * (SB // P) + sk_, dh * SB:(dh + 1) * SB])
                nc.gpsimd.collective_compute(
                    "AllGather", mybir.AluOpType.bypass, replica_groups=RG,
                    ins=[kvin[jb].opt()], outs=[kvout[jb].opt()])

            # ---------- Phase 1: Q^T projection + RoPE (overlaps the AGs) ---
            WqRes = wres.tile([P, NT, D], BF16, tag="wqo")
            for t in range(NT):
                for hh in range(2):
                    w_sl = slice(hh * SB, (hh + 1) * SB)
                    nc.sync.dma_start(WqRes[:, t, w_sl], WqT_r[:, t, w_sl])
            xqs = []
            for qh in range(2):
                sl = slice(qh * SB, (qh + 1) * SB)
                xq = xbp.tile([P, NT, SB], BF16, tag="xb")
                for t in range(NT):
                    # qh1's slot recycles xm1 (read until the last V-proj
                    # group); scalar-issue it late so its waiting
                    # descriptors don't camp in the DMA queue FIFOs
                    if qh == 0:
                        nc.sync.dma_start(xq[:, t, :], xTq_r[:, t, sl])
                    else:
                        nc.scalar.dma_start(xq[:, t, :], xTq_r[:, t, sl])
                xqs.append(xq)

            for qh in range(2):
                sl = slice(qh * SB, (qh + 1) * SB)
                xq = xqs[qh]
                # scalar-issued so these don't queue behind the collective
                # triggers sitting on the gpsimd queue
                cq = csp.tile([P, NT // 2, SB], BF16, tag="cs_c")
                nc.scalar.dma_start(cq[:], cosQ_r[:, :, sl])
                sq = csp.tile([P, NT // 2, SB], BF16, tag="cs_s")
                nc.scalar.dma_start(sq[:], sinQ_r[:, :, sl])
                qraw = rawp.tile([P, NT, SB], BF16, tag="raw")
                for t_out in range(NT):
                    ps = psB.tile([P, SB], F32, tag="psB")
                    for dt_ in range(NT):
                        nc.tensor.matmul(ps[:], WqRes[:, dt_, t_out * P:(t_out + 1) * P],
                                         xq[:, dt_, :], start=(dt_ == 0), stop=(dt_ == NT - 1))
                    nc.vector.tensor_copy(qraw[:, t_out, :], ps[:])
                rope_block(QT[:, :, sl], qraw, cq, sq)

            # Gathered K/V blocks stream back as soon as each AG lands.
            for jb in range(2):
                for r in range(2):
                    g = 2 * jb + r
                    gsl = slice(g * SB, (g + 1) * SB)
                    for t in range(NT):
                        nc.gpsimd.dma_start(KT[:, t, gsl], kvout[jb][r, :, t, :])
                    for sk_ in range(SB // P):
                        for dh in range(2):
                            nc.gpsimd.dma_start(V[:, g * (SB // P) + sk_, dh * SB:(dh + 1) * SB],
                                                kvout[jb][r, :, 8 + 2 * sk_ + dh, :])
            # Wo shares the WqRes slot; issued last (needed only at the end).
            WoRes = wres.tile([P, NT, D], BF16, tag="wqo")
            for t in range(NT):
                nc.sync.dma_start(WoRes[:, t, :], WoT_r[:, t, :])

            # ---------- attention slots ----------
            for sb in range(S // SB):
                # wave B of the previous slot (overlaps this slot's scores)
                if sb > 0:
                    pc, pqsl, pPT, pbc = prev
                    pvB = [psPV.tile([P, SB], F32, tag="pv", name=f"pvB{sb}_{j2}")
                           for j2 in range(4)]
                    for v in range(pc):
                        for j2 in range(4):
                            nc.tensor.matmul(pvB[j2][:, 0:QB],
                                             V[:, v, (4 + j2) * P:(5 + j2) * P],
                                             pPT[:, v, :], start=(v == 0),
                                             stop=(v == pc - 1))
                    for j2 in range(4):
                        nc.vector.tensor_mul(out=attnT[:, 4 + j2, pqsl],
                                             in0=pvB[j2][:, 0:QB], in1=pbc[:])

                s = sb
                c = NVIS[s]
                qsl = slice(s * QB, (s + 1) * QB)
                sumacc = sap.tile([P, QB], F32R, tag="sa")
                PT = ptp.tile([P, 16, QB], BF16, tag="pts")
                pvA = [psPV.tile([P, SB], F32, tag="pv", name=f"pvA{s}_{j2}")
                       for j2 in range(4)]
                for v in range(c):
                    ps = psB.tile([P, SB], F32, tag="psB")
                    for dt_ in range(NT):
                        nc.tensor.matmul(ps[:, 0:QB], KT[:, dt_, v * P:(v + 1) * P],
                                         QT[:, dt_, qsl], start=(dt_ == 0), stop=(dt_ == NT - 1))
                    nc.scalar.activation(PT[:, v, :], ps[:, 0:QB],
                                         mybir.ActivationFunctionType.Exp, scale=SCALE)
                    if v >= c - 4:
                        nc.vector.tensor_mul(out=PT[:, v, :], in0=PT[:, v, :],
                                             in1=maskst[:, 4 * s + (v - (c - 4)), :])
                    if v == 0:
                        nc.vector.tensor_copy(sumacc[:], PT[:, v, :])
                    else:
                        nc.vector.tensor_tensor(sumacc[:], sumacc[:], PT[:, v, :],
                                                mybir.AluOpType.add)
                    if v > 0:
                        for j2 in range(4):
                            nc.tensor.matmul(pvA[j2][:, 0:QB],
                                             V[:, v - 1, j2 * P:(j2 + 1) * P], PT[:, v - 1, :],
                                             start=(v - 1 == 0), stop=False)
                for j2 in range(4):
                    nc.tensor.matmul(pvA[j2][:, 0:QB],
                                     V[:, c - 1, j2 * P:(j2 + 1) * P], PT[:, c - 1, :],
                                     start=(c == 1), stop=True)
                # normalize: 1/rowsum broadcast via PE, scale wave-A chunks
                sums_ps = psB.tile([P, SB], F32, tag="psB")
                nc.tensor.matmul(sums_ps[0:1, 0:QB], ones_col[:], sumacc[:],
                                 start=True, stop=True)
                sumrow = smp.tile([1, QB], F32R, tag="sumrow")
                nc.scalar.copy(sumrow[:], sums_ps[0:1, 0:QB])
                bc_ps = psB.tile([P, SB], F32, tag="psB")
                nc.tensor.matmul(bc_ps[:, 0:QB], ones_row[:], sumrow[:],
                                 start=True, stop=True)
                bc_sums = bcp.tile([P, QB], F32R, tag="bcs")
                nc.scalar.copy(bc_sums[:], bc_ps[:, 0:QB])
                bc = bcp.tile([P, QB], F32, tag="bc")
                nc.vector.reciprocal(bc[:], bc_sums[:])
                for j2 in range(4):
                    nc.vector.tensor_mul(out=attnT[:, j2, qsl],
                                         in0=pvA[j2][:, 0:QB], in1=bc[:])
                prev = (c, qsl, PT, bc)

            # ---------- wave B of the last slot ----------
            pc, pqsl, pPT, pbc = prev
            pvB = [psPV.tile([P, SB], F32, tag="pv", name=f"pvBf_{j}")
                   for j in range(4)]
            for v in range(pc):
                for j in range(4):
                    nc.tensor.matmul(pvB[j][:, 0:QB],
                                     V[:, v, (4 + j) * P:(5 + j) * P],
                                     pPT[:, v, :], start=(v == 0), stop=(v == pc - 1))
            for j in range(4):
                nc.vector.tensor_mul(out=attnT[:, 4 + j, pqsl],
                                     in0=pvB[j][:, 0:QB], in1=pbc[:])

            # ---------- output projection ----------
            for qh in range(4):
                sl = slice(qh * QB, (qh + 1) * QB)
                ob = obp.tile([P, NT, QB], BF16, tag="ob")
                for oc in range(NT):
                    ps = psB.tile([P, SB], F32, tag="psB")
                    for dt_ in range(NT):
                        nc.tensor.matmul(ps[:, 0:QB], WoRes[:, dt_, oc * P:(oc + 1) * P],
                                         attnT[:, dt_, sl], start=(dt_ == 0), stop=(dt_ == NT - 1))
                    nc.scalar.copy(ob[:, oc, :], ps[:, 0:QB])
                    nc.sync.dma_start(outT[:, oc, sl], ob[:, oc, :])

    nc.finalize()
    return nc


def _host_inputs(x, Wq, Wk, Wv, Wo, token_positions):
    import ml_dtypes
    bf = ml_dtypes.bfloat16
    perm = np.concatenate([np.arange(0, D, 2), np.arange(1, D, 2)])
    WqTp = np.ascontiguousarray(Wq[perm].T).astype(bf)
    WkTp = np.ascontiguousarray(Wk[perm].T).astype(bf)
    WvT = np.ascontiguousarray(Wv.T).astype(bf)
    WoT = np.ascontiguousarray(Wo.T).astype(bf)
    inv_freq = (1.0 / (np.float32(THETA) **
                       (np.arange(0, D, 2, dtype=np.float32) / np.float32(D))))
    ones_col = np.ones((P, 1), np.float32)
    ones_row = np.ones((1, P), np.float32)

    in_maps, metas = [], []
    for b in range(B):
        xT = np.ascontiguousarray(x[b].T).astype(bf)           # [D, S]
        pos = token_positions[b].astype(np.float32)
        ang = (pos[None, :] * inv_freq[:, None]).astype(np.float32)  # [D/2, S]
        cosF = np.cos(ang)
        sinF = np.sin(ang)
        for h in range(2):
            blocks = BLOCKS[h]
            qcols = np.concatenate([np.arange(QB * bs, QB * (bs + 1))
                                    for bs in blocks])
            xTq = np.ascontiguousarray(xT[:, qcols])
            cosQ = np.ascontiguousarray(cosF[:, qcols]).astype(bf)
            sinQ = np.ascontiguousarray(sinF[:, qcols]).astype(bf)
            # my key half: global seq blocks h and 2+h (512 keys each)
            mcols = np.concatenate([np.arange(SB * h, SB * (h + 1)),
                                    np.arange(1024 + SB * h, 1024 + SB * (h + 1))])
            xTm = np.ascontiguousarray(xT[:, mcols])
            cosM = np.ascontiguousarray(cosF[:, mcols]).astype(bf)
            sinM = np.ascontiguousarray(sinF[:, mcols]).astype(bf)
            m = np.zeros((P, 16, QB), dtype=np.float32)
            for s, bs in enumerate(blocks):
                c = NVIS[s]
                q0 = QB * bs
                q_glob = q0 + np.arange(QB)
                for j in range(4):
                    v = c - 4 + j
                    k_glob = 128 * v + np.arange(P)
                    m[:, 4 * s + j, :] = (q_glob[None, :] >= k_glob[:, None])
            in_maps.append({
                "ones_col": ones_col, "ones_row": ones_row,
                "xTm": xTm, "xTq": xTq,
                "WqT": WqTp, "WkT": WkTp, "WvT": WvT, "WoT": WoT,
                "cosM": cosM, "sinM": sinM,
                "cosQ": cosQ, "sinQ": sinQ,
                "masks": m.astype(bf),
            })
            metas.append((b, qcols))
    return in_maps, metas


_NC_CACHE = {}


def kernel(x, Wq, Wk, Wv, Wo, token_positions):
    x = np.asarray(x); token_positions = np.asarray(token_positions)
    if "nc" not in _NC_CACHE:
        _NC_CACHE["nc"] = _build_program()
    nc = _NC_CACHE["nc"]
    in_maps, metas = _host_inputs(np.asarray(x), np.asarray(Wq), np.asarray(Wk),
                                  np.asarray(Wv), np.asarray(Wo), token_positions)
    res = run_bass_kernel_spmd(nc, in_maps, core_ids=list(range(8)))
    out = np.empty((B, S, D), dtype=np.float32)
    for (b, qcols), r in zip(metas, res.results):
        oT = np.asarray(r["outT"]).astype(np.float32)   # [P, NT, NQ]
        o = np.transpose(oT, (2, 1, 0)).reshape(NQ, D)
        out[b, qcols, :] = o
    return out


# revision 19
# speedup vs baseline: 1.0273x; 1.0273x over previous
"""Causal no-head self-attention with RoPE on 8 Trainium2 NeuronCores.

Sharding: 8 cores = 4 batches x 2 query-sets of four 256-query blocks.
Per-core slots s=0..3 run 4(s+1) key-tile visits (128 keys each); block
assignment (h=0: blocks {1,3,4,7}, h=1: {0,2,5,6}) makes the same
(4,8,12,16) visit structure causally sufficient on every core, so all
cores run ONE identical Bass program; per-core differences (which
queries, which keys, causal masks, RoPE angles) are carried in the
input data.

K/V projection is deduplicated across the two cores sharing a batch:
core h projects (and RoPEs) keys [512h,512h+512) u [1024+512h,...+512)
only, and the pair exchanges 512-key blocks with two in-pair
AllGathers (AG#0 -> seq blocks 0,1; AG#1 -> blocks 2,3) that overlap
the Q projection. Everything is bf16 (PE full rate, half DMA/SBUF of
fp32) with fp32 PSUM accumulation; K^T/V/Q^T live entirely in SBUF.
QT/KT are [d_k, cols] with d_k de-interleaved (even feats rows 0:512,
odd 512:1024) so RoPE is a contiguous-partition-block rotation; the
same permutation is applied to Wq/Wk output columns on host (scores
are invariant). QT doubles as attnT storage.
"""

import numpy as np
import sys

for _p in ("/opt/trn_rl_repo",):
    if _p not in sys.path:
        sys.path.insert(0, _p)

import concourse.bass as bass
import concourse.bacc as bacc
import concourse.mybir as mybir
from concourse.tile import TileContext
from concourse.bass_utils import run_bass_kernel_spmd

B, S, D = 4, 2048, 1024
THETA = 10000.0
P = 128
NT = D // P          # 8 partition-tiles over the 1024 dim
SB = 512             # seq block width for K/V projection
QB = 256             # query slot width (4 slots per core)
NQ = 1024            # queries (and local keys) per core
F32 = mybir.dt.float32
F32R = mybir.dt.float32r
BF16 = mybir.dt.bfloat16
SCALE = 1.0 / 32.0   # 1/sqrt(D)
BLOCKS = [[1, 3, 4, 7], [0, 2, 5, 6]]   # 256-query blocks per core half
NVIS = [4, 8, 12, 16]                    # kt visits per slot (same all cores)


def _build_program():
    nc = bacc.Bacc("TRN2", num_swdge_queues=4)
    inp = {}
    def din(name, shape, dt):
        inp[name] = nc.dram_tensor(name, shape, dt, kind="ExternalInput")
    din("xTm", [D, NQ], BF16)
    din("xTq", [D, NQ], BF16)
    din("WqT", [D, D], BF16)
    din("WkT", [D, D], BF16)
    din("WvT", [D, D], BF16)
    din("WoT", [D, D], BF16)
    din("cosM", [D // 2, NQ], BF16)
    din("sinM", [D // 2, NQ], BF16)
    din("cosQ", [D // 2, NQ], BF16)
    din("sinQ", [D // 2, NQ], BF16)
    din("masks", [P, 16, QB], BF16)
    din("ones_col", [P, 1], F32R)
    din("ones_row", [1, P], F32R)
    outT = nc.dram_tensor("outT", [P, NT, NQ], BF16, kind="ExternalOutput")

    xTm_r = inp["xTm"].rearrange("(t p) s -> p t s", p=P)
    xTq_r = inp["xTq"].rearrange("(t p) s -> p t s", p=P)
    WqT_r = inp["WqT"].rearrange("(t p) o -> p t o", p=P)
    WkT_r = inp["WkT"].rearrange("(t p) o -> p t o", p=P)
    WvT_r = inp["WvT"].rearrange("(t p) o -> p t o", p=P)
    WoT_r = inp["WoT"].rearrange("(t p) o -> p t o", p=P)
    cosM_r = inp["cosM"].rearrange("(t p) s -> p t s", p=P)
    sinM_r = inp["sinM"].rearrange("(t p) s -> p t s", p=P)
    cosQ_r = inp["cosQ"].rearrange("(t p) s -> p t s", p=P)
    sinQ_r = inp["sinQ"].rearrange("(t p) s -> p t s", p=P)

    from contextlib import ExitStack
    with TileContext(nc) as tc:
        with ExitStack() as ctx:
            pool = lambda *a, **kw: ctx.enter_context(tc.tile_pool(*a, **kw))
            res = pool(name="res", bufs=1)          # big residents
            dpool = pool(name="dram", bufs=1, space="DRAM")
            wres = pool(name="wres", bufs=1)        # weights
            smp = pool(name="small", bufs=1)
            xbp = pool(name="xb", bufs=2)
            csp = pool(name="cs", bufs=2)
            rawp = pool(name="raw", bufs=2)
            tmpp = pool(name="tmp", bufs=1)
            ptp = pool(name="pt", bufs=1)
            sap = pool(name="sa", bufs=1)
            bcp = pool(name="bc", bufs=2)
            obp = pool(name="ob", bufs=1)
            psB = pool(name="psB", bufs=3, space="PSUM")
            psPV = pool(name="psPV", bufs=4, space="PSUM")

            KT = res.tile([P, NT, S], BF16, tag="kt")
            V = res.tile([P, S // P, D], BF16, tag="v")
            # QT doubles as attnT: slot s's normalized PV overwrites QT's
            # columns after the slot's scores are done reading them.
            QT = res.tile([P, NT, NQ], BF16, tag="qt")
            attnT = QT
            maskst = res.tile([P, 16, QB], BF16, tag="msk")
            ones_col = smp.tile([P, 1], F32R, tag="onescol")
            nc.sync.dma_start(ones_col[:], inp["ones_col"][:])
            ones_row = smp.tile([1, P], F32R, tag="onesrow")
            nc.sync.dma_start(ones_row[:], inp["ones_row"][:])

            def rope_block(dst, src, cos_t, sin_t):
                # dst/src: [P, NT, w]; rows 0:NT/2 = even feats, NT/2: = odd
                h = NT // 2
                w = src.shape[-1]
                e, o = src[:, 0:h, :], src[:, h:NT, :]
                c, s = cos_t[:, :, :], sin_t[:, :, :]
                t1 = tmpp.tile([P, h, SB], BF16, tag="t1")
                nc.vector.tensor_mul(out=dst[:, 0:h, :], in0=e, in1=c)
                nc.vector.tensor_mul(out=t1[:, :, 0:w], in0=o, in1=s)
                nc.vector.tensor_tensor(dst[:, 0:h, :], dst[:, 0:h, :],
                                        t1[:, :, 0:w], mybir.AluOpType.subtract)
                t2 = tmpp.tile([P, h, SB], BF16, tag="t1")
                nc.vector.tensor_mul(out=dst[:, h:NT, :], in0=o, in1=c)
                nc.vector.tensor_mul(out=t2[:, :, 0:w], in0=e, in1=s)
                nc.vector.tensor_tensor(dst[:, h:NT, :], dst[:, h:NT, :],
                                        t2[:, :, 0:w], mybir.AluOpType.add)

            # ---------- Phase 0: K/V projection of MY half of the keys ------
            # jb-th local block = global seq block 2*jb + rank; the in-pair
            # AllGather over [rank0 chunk | rank1 chunk] then yields seq
            # blocks {0,1} (AG#0) and {2,3} (AG#1) in natural order.
            RG = [[0, 1], [2, 3], [4, 5], [6, 7]]
            kvin = [dpool.tile([P, 16, SB], BF16, name=f"kvin{j}") for j in range(2)]
            kvout = [dpool.tile([2, P, 16, SB], BF16, name=f"kvout{j}") for j in range(2)]

            xms, cms, sms = [], [], []
            for jb in range(2):
                sl = slice(jb * SB, (jb + 1) * SB)
                xm = xbp.tile([P, NT, SB], BF16, tag="xb")
                for t in range(NT):
                    nc.sync.dma_start(xm[:, t, :], xTm_r[:, t, sl])
                cm = csp.tile([P, NT // 2, SB], BF16, tag="cs_c")
                nc.gpsimd.dma_start(cm[:], cosM_r[:, :, sl])
                sm = csp.tile([P, NT // 2, SB], BF16, tag="cs_s")
                nc.gpsimd.dma_start(sm[:], sinM_r[:, :, sl])
                xms.append(xm); cms.append(cm); sms.append(sm)
                if jb == 0:
                    WkRes = wres.tile([P, NT, D], BF16, tag="wk")
                    for t in range(NT):
                        for hh in range(2):
                            w_sl = slice(hh * SB, (hh + 1) * SB)
                            nc.sync.dma_start(WkRes[:, t, w_sl], WkT_r[:, t, w_sl])
                else:
                    WvRes = wres.tile([P, NT, D], BF16, tag="wv")
                    for t in range(NT):
                        for hh in range(2):
                            w_sl = slice(hh * SB, (hh + 1) * SB)
                            nc.sync.dma_start(WvRes[:, t, w_sl], WvT_r[:, t, w_sl])
            nc.gpsimd.dma_start(maskst[:], inp["masks"][:])

            for jb in range(2):
                sl = slice(jb * SB, (jb + 1) * SB)
                xb = xms[jb]
                kraw = rawp.tile([P, NT, SB], BF16, tag="raw")
                for t_out in range(NT):
                    ps = psB.tile([P, SB], F32, tag="psB")
                    for dt_ in range(NT):
                        nc.tensor.matmul(ps[:], WkRes[:, dt_, t_out * P:(t_out + 1) * P],
                                         xb[:, dt_, :], start=(dt_ == 0), stop=(dt_ == NT - 1))
                    nc.vector.tensor_copy(kraw[:, t_out, :], ps[:])
                rope_block(KT[:, :, sl], kraw, cms[jb], sms[jb])
                for sk_ in range(SB // P):
                    for dh in range(2):
                        ps = psB.tile([P, SB], F32, tag="psB")
                        for dt_ in range(NT):
                            nc.tensor.matmul(ps[:], xb[:, dt_, sk_ * P:(sk_ + 1) * P],
                                             WvRes[:, dt_, dh * SB:(dh + 1) * SB],
                                             start=(dt_ == 0), stop=(dt_ == NT - 1))
                        nc.scalar.copy(V[:, jb * (SB // P) + sk_, dh * SB:(dh + 1) * SB], ps[:])
                # spills issued from scalar right after the V copies, when
                # all the data is already resident -- the descriptors reach
                # the DMA queues without blocking them on waits
                for t in range(NT):
                    nc.scalar.dma_start(kvin[jb][:, t, :], KT[:, t, sl])
                for sk_ in range(SB // P):
                    for dh in range(2):
                        nc.scalar.dma_start(kvin[jb][:, 8 + 2 * sk_ + dh, :],
                                            V[:, jb * (SB // P) + sk_, dh * SB:(dh + 1) * SB])
                nc.gpsimd.collective_compute(
                    "AllGather", mybir.AluOpType.bypass, replica_groups=RG,
                    ins=[kvin[jb].opt()], outs=[kvout[jb].opt()])

            # ---------- Phase 1: Q^T projection + RoPE (overlaps the AGs) ---
            WqRes = wres.tile([P, NT, D], BF16, tag="wqo")
            for t in range(NT):
                for hh in range(2):
                    w_sl = slice(hh * SB, (hh + 1) * SB)
                    nc.sync.dma_start(WqRes[:, t, w_sl], WqT_r[:, t, w_sl])
            xqs = []
            for qh in range(2):
                sl = slice(qh * SB, (qh + 1) * SB)
                xq = xbp.tile([P, NT, SB], BF16, tag="xb")
                for t in range(NT):
                    nc.sync.dma_start(xq[:, t, :], xTq_r[:, t, sl])
                xqs.append(xq)

            for qh in range(2):
                sl = slice(qh * SB, (qh + 1) * SB)
                xq = xqs[qh]
                # scalar-issued so these don't queue behind the collective
                # triggers sitting on the gpsimd queue
                cq = csp.tile([P, NT // 2, SB], BF16, tag="cs_c")
                nc.scalar.dma_start(cq[:], cosQ_r[:, :, sl])
                sq = csp.tile([P, NT // 2, SB], BF16, tag="cs_s")
                nc.scalar.dma_start(sq[:], sinQ_r[:, :, sl])
                qraw = rawp.tile([P, NT, SB], BF16, tag="raw")
                for t_out in range(NT):
                    ps = psB.tile([P, SB], F32, tag="psB")
                    for dt_ in range(NT):
                        nc.tensor.matmul(ps[:], WqRes[:, dt_, t_out * P:(t_out + 1) * P],
                                         xq[:, dt_, :], start=(dt_ == 0), stop=(dt_ == NT - 1))
                    nc.vector.tensor_copy(qraw[:, t_out, :], ps[:])
                rope_block(QT[:, :, sl], qraw, cq, sq)

            # Gathered K/V blocks stream back as soon as each AG lands.
            for jb in range(2):
                for r in range(2):
                    g = 2 * jb + r
                    gsl = slice(g * SB, (g + 1) * SB)
                    for t in range(NT):
                        nc.gpsimd.dma_start(KT[:, t, gsl], kvout[jb][r, :, t, :])
                    for sk_ in range(SB // P):
                        for dh in range(2):
                            nc.gpsimd.dma_start(V[:, g * (SB // P) + sk_, dh * SB:(dh + 1) * SB],
                                                kvout[jb][r, :, 8 + 2 * sk_ + dh, :])
            # Wo shares the WqRes slot; issued last (needed only at the end).
            WoRes = wres.tile([P, NT, D], BF16, tag="wqo")
            for t in range(NT):
                nc.sync.dma_start(WoRes[:, t, :], WoT_r[:, t, :])

            # ---------- attention slots ----------
            for sb in range(S // SB):
                # wave B of the previous slot (overlaps this slot's scores)
                if sb > 0:
                    pc, pqsl, pPT, pbc = prev
                    pvB = [psPV.tile([P, SB], F32, tag="pv", name=f"pvB{sb}_{j2}")
                           for j2 in range(4)]
                    for v in range(pc):
                        for j2 in range(4):
                            nc.tensor.matmul(pvB[j2][:, 0:QB],
                                             V[:, v, (4 + j2) * P:(5 + j2) * P],
                                             pPT[:, v, :], start=(v == 0),
                                             stop=(v == pc - 1))
                    for j2 in range(4):
                        nc.vector.tensor_mul(out=attnT[:, 4 + j2, pqsl],
                                             in0=pvB[j2][:, 0:QB], in1=pbc[:])

                s = sb
                c = NVIS[s]
                qsl = slice(s * QB, (s + 1) * QB)
                sumacc = sap.tile([P, QB], F32R, tag="sa")
                PT = ptp.tile([P, 16, QB], BF16, tag="pts")
                pvA = [psPV.tile([P, SB], F32, tag="pv", name=f"pvA{s}_{j2}")
                       for j2 in range(4)]
                for v in range(c):
                    ps = psB.tile([P, SB], F32, tag="psB")
                    for dt_ in range(NT):
                        nc.tensor.matmul(ps[:, 0:QB], KT[:, dt_, v * P:(v + 1) * P],
                                         QT[:, dt_, qsl], start=(dt_ == 0), stop=(dt_ == NT - 1))
                    nc.scalar.activation(PT[:, v, :], ps[:, 0:QB],
                                         mybir.ActivationFunctionType.Exp, scale=SCALE)
                    if v >= c - 4:
                        nc.vector.tensor_mul(out=PT[:, v, :], in0=PT[:, v, :],
                                             in1=maskst[:, 4 * s + (v - (c - 4)), :])
                    if v == 0:
                        nc.vector.tensor_copy(sumacc[:], PT[:, v, :])
                    else:
                        nc.vector.tensor_tensor(sumacc[:], sumacc[:], PT[:, v, :],
                                                mybir.AluOpType.add)
                    if v > 0:
                        for j2 in range(4):
                            nc.tensor.matmul(pvA[j2][:, 0:QB],
                                             V[:, v - 1, j2 * P:(j2 + 1) * P], PT[:, v - 1, :],
                                             start=(v - 1 == 0), stop=False)
                for j2 in range(4):
                    nc.tensor.matmul(pvA[j2][:, 0:QB],
                                     V[:, c - 1, j2 * P:(j2 + 1) * P], PT[:, c - 1, :],
                                     start=(c == 1), stop=True)
                # normalize: 1/rowsum broadcast via PE, scale wave-A chunks
                sums_ps = psB.tile([P, SB], F32, tag="psB")
                nc.tensor.matmul(sums_ps[0:1, 0:QB], ones_col[:], sumacc[:],
                                 start=True, stop=True)
                sumrow = smp.tile([1, QB], F32R, tag="sumrow")
                nc.scalar.copy(sumrow[:], sums_ps[0:1, 0:QB])
                bc_ps = psB.tile([P, SB], F32, tag="psB")
                nc.tensor.matmul(bc_ps[:, 0:QB], ones_row[:], sumrow[:],
                                 start=True, stop=True)
                bc_sums = bcp.tile([P, QB], F32R, tag="bcs")
                nc.scalar.copy(bc_sums[:], bc_ps[:, 0:QB])
                bc = bcp.tile([P, QB], F32, tag="bc")
                nc.vector.reciprocal(bc[:], bc_sums[:])
                for j2 in range(4):
                    nc.vector.tensor_mul(out=attnT[:, j2, qsl],
                                         in0=pvA[j2][:, 0:QB], in1=bc[:])
                prev = (c, qsl, PT, bc)

            # ---------- wave B of the last slot ----------
            pc, pqsl, pPT, pbc = prev
            pvB = [psPV.tile([P, SB], F32, tag="pv", name=f"pvBf_{j}")
                   for j in range(4)]
            for v in range(pc):
                for j in range(4):
                    nc.tensor.matmul(pvB[j][:, 0:QB],
                                     V[:, v, (4 + j) * P:(5 + j) * P],
                                     pPT[:, v, :], start=(v == 0), stop=(v == pc - 1))
            for j in range(4):
                nc.vector.tensor_mul(out=attnT[:, 4 + j, pqsl],
                                     in0=pvB[j][:, 0:QB], in1=pbc[:])

            # ---------- output projection ----------
            for qh in range(4):
                sl = slice(qh * QB, (qh + 1) * QB)
                ob = obp.tile([P, NT, QB], BF16, tag="ob")
                for oc in range(NT):
                    ps = psB.tile([P, SB], F32, tag="psB")
                    for dt_ in range(NT):
                        nc.tensor.matmul(ps[:, 0:QB], WoRes[:, dt_, oc * P:(oc + 1) * P],
                                         attnT[:, dt_, sl], start=(dt_ == 0), stop=(dt_ == NT - 1))
                    nc.scalar.copy(ob[:, oc, :], ps[:, 0:QB])
                    nc.sync.dma_start(outT[:, oc, sl], ob[:, oc, :])

    nc.finalize()
    return nc


def _host_inputs(x, Wq, Wk, Wv, Wo, token_positions):
    import ml_dtypes
    bf = ml_dtypes.bfloat16
    perm = np.concatenate([np.arange(0, D, 2), np.arange(1, D, 2)])
    WqTp = np.ascontiguousarray(Wq[perm].T).astype(bf)
    WkTp = np.ascontiguousarray(Wk[perm].T).astype(bf)
    WvT = np.ascontiguousarray(Wv.T).astype(bf)
    WoT = np.ascontiguousarray(Wo.T).astype(bf)
    inv_freq = (1.0 / (np.float32(THETA) **
                       (np.arange(0, D, 2, dtype=np.float32) / np.float32(D))))
    ones_col = np.ones((P, 1), np.float32)
    ones_row = np.ones((1, P), np.float32)

    in_maps, metas = [], []
    for b in range(B):
        xT = np.ascontiguousarray(x[b].T).astype(bf)           # [D, S]
        pos = token_positions[b].astype(np.float32)
        ang = (pos[None, :] * inv_freq[:, None]).astype(np.float32)  # [D/2, S]
        cosF = np.cos(ang)
        sinF = np.sin(ang)
        for h in range(2):
            blocks = BLOCKS[h]
            qcols = np.concatenate([np.arange(QB * bs, QB * (bs + 1))
                                    for bs in blocks])
            xTq = np.ascontiguousarray(xT[:, qcols])
            cosQ = np.ascontiguousarray(cosF[:, qcols]).astype(bf)
            sinQ = np.ascontiguousarray(sinF[:, qcols]).astype(bf)
            # my key half: global seq blocks h and 2+h (512 keys each)
            mcols = np.concatenate([np.arange(SB * h, SB * (h + 1)),
                                    np.arange(1024 + SB * h, 1024 + SB * (h + 1))])
            xTm = np.ascontiguousarray(xT[:, mcols])
            cosM = np.ascontiguousarray(cosF[:, mcols]).astype(bf)
            sinM = np.ascontiguousarray(sinF[:, mcols]).astype(bf)
            m = np.zeros((P, 16, QB), dtype=np.float32)
            for s, bs in enumerate(blocks):
                c = NVIS[s]
                q0 = QB * bs
                q_glob = q0 + np.arange(QB)
                for j in range(4):
                    v = c - 4 + j
                    k_glob = 128 * v + np.arange(P)
                    m[:, 4 * s + j, :] = (q_glob[None, :] >= k_glob[:, None])
            in_maps.append({
                "ones_col": ones_col, "ones_row": ones_row,
                "xTm": xTm, "xTq": xTq,
                "WqT": WqTp, "WkT": WkTp, "WvT": WvT, "WoT": WoT,
                "cosM": cosM, "sinM": sinM,
                "cosQ": cosQ, "sinQ": sinQ,
                "masks": m.astype(bf),
            })
            metas.append((b, qcols))
    return in_maps, metas


_NC_CACHE = {}


def kernel(x, Wq, Wk, Wv, Wo, token_positions):
    x = np.asarray(x); token_positions = np.asarray(token_positions)
    if "nc" not in _NC_CACHE:
        _NC_CACHE["nc"] = _build_program()
    nc = _NC_CACHE["nc"]
    in_maps, metas = _host_inputs(np.asarray(x), np.asarray(Wq), np.asarray(Wk),
                                  np.asarray(Wv), np.asarray(Wo), token_positions)
    res = run_bass_kernel_spmd(nc, in_maps, core_ids=list(range(8)))
    out = np.empty((B, S, D), dtype=np.float32)
    for (b, qcols), r in zip(metas, res.results):
        oT = np.asarray(r["outT"]).astype(np.float32)   # [P, NT, NQ]
        o = np.transpose(oT, (2, 1, 0)).reshape(NQ, D)
        out[b, qcols, :] = o
    return out
